# revision 22
# baseline (speedup 1.0000x reference)
"""
Trainium2 Bass kernel for nn_BMM_S8T_S8N_S8T:
  y[b,m,n] = sat_i8(round(alpha * sum_k a[b,m,k] * b[b,n,k]))
with a,b int8 [128, 1024, 128], alpha scalar.

Strategy (8 NeuronCores, batch-parallel, 16 batches/core):
 - Host: pre-transpose a -> [BPC, K, M], b -> [BPC, K, N] so SBUF tiles land
   directly in [contraction-partition, free] layout. No on-chip transposes.
 - Input DMA on SWDGE (gpsimd) casts int8 -> bf16 in the DMA datapath; the
   engines never touch input conversion. bf16 holds int8 exactly; products
   (<= 2^14) and fp32 accumulations (|acc| <= 2^21) are bit-exact.
 - Matmuls: per batch, 8 stationary A-tiles [128k, 128m] x moving B [128k, 512n]
   pairs into [128, 2048] fp32 PSUM tiles (4 banks, double-buffered).
 - Epilogue: one op per PSUM tile: int8 out = rne_sat(alpha*acc), alternating
   ACT (activation Copy w/ scale) and DVE (tensor_scalar mult) in a 5:4
   pattern that balances the 1.2 GHz vs 0.96 GHz engines. This drain is the
   critical path (~64 us); everything else hides under it.
 - Stores: one 1 MiB DMA per batch, alternating the two HWDGE rings.
"""

import sys

sys.path.insert(0, "/opt/trn_rl_repo")

import numpy as np

N_CORES = 8
B, M, N, K = 128, 1024, 1024, 128
BPC = B // N_CORES  # batches per core
MT = M // 128
HALF = BPC // 2
NBF = 4  # leading batches shipped as host-prepared bf16 (prologue fast path)

_cache = {}


def _build(alpha: float):
    import concourse.bacc as bacc
    import concourse.tile as tile
    import concourse.mybir as mybir

    nc = bacc.Bacc("TRN2", target_bir_lowering=False, debug=False)

    a_t = nc.dram_tensor("a_t", [BPC, K, M], mybir.dt.int8, kind="ExternalInput")
    b_t = nc.dram_tensor("b_t", [BPC, K, N], mybir.dt.int8, kind="ExternalInput")
    # host-prepared bf16 copies of the first NBF batches (prologue fast path:
    # HWDGE loads them directly, engines never do input conversion)
    a_bf = nc.dram_tensor("a_bf", [NBF, K, M], mybir.dt.bfloat16, kind="ExternalInput")
    b_bf = nc.dram_tensor("b_bf", [NBF, K, N], mybir.dt.bfloat16, kind="ExternalInput")
    y = nc.dram_tensor("y", [BPC, M, N], mybir.dt.int8, kind="ExternalOutput")

    bf16 = mybir.dt.bfloat16
    f32 = mybir.dt.float32
    i8 = mybir.dt.int8

    a_v = a_t.rearrange("b k m -> k b m")  # [128, BPC, 1024]
    b_v = b_t.rearrange("b k n -> k b n")

    with tile.TileContext(nc) as tc:
        with (
            tc.tile_pool(name="inp", bufs=1) as ipool,
            tc.tile_pool(name="outp", bufs=10) as opool,
            tc.tile_pool(name="ps", bufs=4, space="PSUM") as pspool,
        ):
            # input tiles: all 16 batches resident as bf16 (64 KB/partition).
            # One tile per DMA writer: a tile written by multiple DMAs gets a
            # coarse "all writers done" readiness sem that stalls consumers
            # needing only the first writer.
            a_host, b_host = [], []
            for i in range(NBF):
                th_a = ipool.tile([128, M], bf16, tag=f"ah{i}")
                th_b = ipool.tile([128, N], bf16, tag=f"bh{i}")
                a_host.append(th_a)
                b_host.append(th_b)
            a_mid = ipool.tile([128, 8 - NBF, M], bf16, tag="amid")
            b_mid = ipool.tile([128, 8 - NBF, N], bf16, tag="bmid")
            a_hi = ipool.tile([128, HALF, M], bf16, tag="ahi")
            b_hi = ipool.tile([128, HALF, N], bf16, tag="bhi")

            def a_of(bi):
                if bi < NBF:
                    return a_host[bi][:]
                if bi < 8:
                    return a_mid[:, bi - NBF, :]
                return a_hi[:, bi - 8, :]

            def b_of(bi):
                if bi < NBF:
                    return b_host[bi][:]
                if bi < 8:
                    return b_mid[:, bi - NBF, :]
                return b_hi[:, bi - 8, :]

            # PE warm-up: ~3.4us of dummy matmuls on a zeroed tile so the
            # HAM clock-gate un-throttles before the first real matmul lands.
            wrm = ipool.tile([128, 640], bf16, tag="wrm")
            nc.vector.memset(wrm[:], 0.0)
            ps0 = pspool.tile([128, 2, 512], f32, tag="ps")
            for w in range(8):
                nc.tensor.matmul(
                    ps0[:, w % 2, :],
                    wrm[:, 0:128],
                    wrm[:, 128:640],
                    start=True,
                    stop=True,
                )

            # Batches 0..NBF-1: host-prepared bf16 via the two HWDGE rings in
            # parallel, one DMA per batch so batch 0 lands ASAP (no cast, no
            # engine work). Batches NBF-15: SWDGE cast-DMA (int8->bf16 in the
            # DMA datapath) in 4 big chunks to amortize the ~2.7us per-DMA Q7
            # issue cost.
            for i in range(NBF):
                nc.sync.dma_start(out=a_host[i][:], in_=a_bf[i])
                nc.scalar.dma_start(out=b_host[i][:], in_=b_bf[i])
            nc.gpsimd.dma_start(out=a_mid[:], in_=a_v[:, NBF:8, :])
            nc.gpsimd.dma_start(out=b_mid[:], in_=b_v[:, NBF:8, :])
            nc.gpsimd.dma_start(out=a_hi[:], in_=a_v[:, 8:16, :])
            nc.gpsimd.dma_start(out=b_hi[:], in_=b_v[:, 8:16, :])

            di = 0  # drain-op index for ACT/DVE balancing
            for bi in range(BPC):
                at = a_of(bi)  # [128, 1024] k x m
                bt = b_of(bi)  # [128, 1024] k x n
                y_sb = opool.tile([128, MT, N], i8, tag="y")

                for mt in range(MT):  # one 2-bank psum tile per m-tile
                    ps = pspool.tile([128, 2, 512], f32, tag="ps")
                    for nh in range(2):
                        nc.tensor.matmul(
                            ps[:, nh, :],
                            at[:, mt * 128 : (mt + 1) * 128],
                            bt[:, nh * 512 : (nh + 1) * 512],
                            start=True,
                            stop=True,
                        )
                    dst = y_sb[:, mt, :]
                    src = ps.rearrange("p a b -> p (a b)")
                    # strict A/D interleave with 3 extra ACT tiles: 67 ACT vs
                    # 61 DVE balances measured 1122ns vs 1215ns per-op costs
                    if di % 2 == 0 or di in (21, 63, 105):
                        nc.scalar.activation(
                            out=dst,
                            in_=src,
                            func=mybir.ActivationFunctionType.Copy,
                            scale=float(alpha),
                        )
                    else:
                        nc.vector.tensor_scalar(
                            out=dst,
                            in0=src,
                            scalar1=float(alpha),
                            scalar2=None,
                            op0=mybir.AluOpType.mult,
                        )
                    di += 1
                    # half-batch stores (512 KiB) as soon as each half is
                    # drained, on the otherwise-idle sync ring only. NEVER on
                    # gpsimd: SWDGE descriptor generation thrashes SBUF ports
                    # and slows every concurrent ACT/DVE op ~20%. The last
                    # batch stores ever-finer so the final store after the
                    # last drain is only 128 KiB; mt6's store issues from the
                    # scalar ring in ACT's post-drain shadow.
                    yv = y[bi].rearrange("(t p) n -> p t n", p=128)
                    last = bi == BPC - 1
                    if mt == 3:
                        nc.sync.dma_start(out=yv[:, 0:4, :], in_=y_sb[:, 0:4, :])
                    elif not last and mt == 7:
                        nc.sync.dma_start(out=yv[:, 4:8, :], in_=y_sb[:, 4:8, :])
                    elif last and mt == 5:
                        nc.sync.dma_start(out=yv[:, 4:6, :], in_=y_sb[:, 4:6, :])
                    elif last and mt == 6:
                        nc.scalar.dma_start(out=yv[:, 6, :], in_=y_sb[:, 6, :])
                    elif last and mt == 7:
                        nc.sync.dma_start(out=yv[:, 7, :], in_=y_sb[:, 7, :])

    nc.compile()
    return nc


def _get_nc(alpha: float):
    key = float(alpha)
    if key not in _cache:
        _cache[key] = _build(key)
    return _cache[key]


def _shard_inputs(a, b):
    import ml_dtypes

    # host-side pre-transpose to [B, K, M] / [B, K, N]
    a_t = np.ascontiguousarray(a.transpose(0, 2, 1))
    b_t = np.ascontiguousarray(b.transpose(0, 2, 1))
    maps = []
    for c in range(N_CORES):
        at = a_t[c * BPC : (c + 1) * BPC]
        bt = b_t[c * BPC : (c + 1) * BPC]
        maps.append(
            {
                "a_t": at,
                "b_t": bt,
                "a_bf": at[:NBF].astype(ml_dtypes.bfloat16),
                "b_bf": bt[:NBF].astype(ml_dtypes.bfloat16),
            }
        )
    return maps


def kernel(a, b, alpha):
    from concourse.bass_utils import run_bass_kernel_spmd

    a = np.asarray(a)
    b = np.asarray(b)
    assert a.shape == (B, M, K) and a.dtype == np.int8
    assert b.shape == (B, N, K) and b.dtype == np.int8

    nc = _get_nc(float(alpha))
    in_maps = _shard_inputs(a, b)
    res = run_bass_kernel_spmd(nc, in_maps, list(range(N_CORES)))
    out = np.concatenate([r["y"] for r in res.results], axis=0)
    return out.astype(np.int8)


# revision 23
# speedup vs baseline: 1.0115x; 1.0115x over previous
"""
Trainium2 Bass kernel for nn_BMM_S8T_S8N_S8T:
  y[b,m,n] = sat_i8(round(alpha * sum_k a[b,m,k] * b[b,n,k]))
with a,b int8 [128, 1024, 128], alpha scalar.

Strategy (8 NeuronCores, batch-parallel, 16 batches/core):
 - Host: pre-transpose a -> [BPC, K, M], b -> [BPC, K, N] so SBUF tiles land
   directly in [contraction-partition, free] layout. No on-chip transposes.
 - Input DMA on SWDGE (gpsimd) casts int8 -> bf16 in the DMA datapath; the
   engines never touch input conversion. bf16 holds int8 exactly; products
   (<= 2^14) and fp32 accumulations (|acc| <= 2^21) are bit-exact.
 - Matmuls: per batch, 8 stationary A-tiles [128k, 128m] x moving B [128k, 512n]
   pairs into [128, 2048] fp32 PSUM tiles (4 banks, double-buffered).
 - Epilogue: one op per PSUM tile: int8 out = rne_sat(alpha*acc), alternating
   ACT (activation Copy w/ scale) and DVE (tensor_scalar mult) in a 5:4
   pattern that balances the 1.2 GHz vs 0.96 GHz engines. This drain is the
   critical path (~64 us); everything else hides under it.
 - Stores: one 1 MiB DMA per batch, alternating the two HWDGE rings.
"""

import sys

sys.path.insert(0, "/opt/trn_rl_repo")

import numpy as np

N_CORES = 8
B, M, N, K = 128, 1024, 1024, 128
BPC = B // N_CORES  # batches per core
MT = M // 128
HALF = BPC // 2
NBF = 4  # leading batches shipped as host-prepared bf16 (prologue fast path)

_cache = {}


def _build(alpha: float):
    import concourse.bacc as bacc
    import concourse.tile as tile
    import concourse.mybir as mybir

    nc = bacc.Bacc("TRN2", target_bir_lowering=False, debug=False)

    a_t = nc.dram_tensor("a_t", [BPC, K, M], mybir.dt.int8, kind="ExternalInput")
    b_t = nc.dram_tensor("b_t", [BPC, K, N], mybir.dt.int8, kind="ExternalInput")
    # host-prepared bf16 copies of the first NBF batches (prologue fast path:
    # HWDGE loads them directly, engines never do input conversion)
    a_bf = nc.dram_tensor("a_bf", [NBF, K, M], mybir.dt.bfloat16, kind="ExternalInput")
    b_bf = nc.dram_tensor("b_bf", [NBF, K, N], mybir.dt.bfloat16, kind="ExternalInput")
    y = nc.dram_tensor("y", [BPC, M, N], mybir.dt.int8, kind="ExternalOutput")

    bf16 = mybir.dt.bfloat16
    f32 = mybir.dt.float32
    i8 = mybir.dt.int8

    a_v = a_t.rearrange("b k m -> k b m")  # [128, BPC, 1024]
    b_v = b_t.rearrange("b k n -> k b n")

    with tile.TileContext(nc) as tc:
        with (
            tc.tile_pool(name="inp", bufs=1) as ipool,
            tc.tile_pool(name="outp", bufs=6) as opool,
            tc.tile_pool(name="ps", bufs=4, space="PSUM") as pspool,
        ):
            # input tiles: all 16 batches resident as bf16 (64 KB/partition).
            # One tile per DMA writer: a tile written by multiple DMAs gets a
            # coarse "all writers done" readiness sem that stalls consumers
            # needing only the first writer.
            a_host, b_host = [], []
            for i in range(NBF):
                th_a = ipool.tile([128, M], bf16, tag=f"ah{i}")
                th_b = ipool.tile([128, N], bf16, tag=f"bh{i}")
                a_host.append(th_a)
                b_host.append(th_b)
            a_mid = ipool.tile([128, 8 - NBF, M], bf16, tag="amid")
            b_mid = ipool.tile([128, 8 - NBF, N], bf16, tag="bmid")
            a_hi = ipool.tile([128, HALF, M], bf16, tag="ahi")
            b_hi = ipool.tile([128, HALF, N], bf16, tag="bhi")

            def a_of(bi):
                if bi < NBF:
                    return a_host[bi][:]
                if bi < 8:
                    return a_mid[:, bi - NBF, :]
                return a_hi[:, bi - 8, :]

            def b_of(bi):
                if bi < NBF:
                    return b_host[bi][:]
                if bi < 8:
                    return b_mid[:, bi - NBF, :]
                return b_hi[:, bi - 8, :]

            # PE warm-up: ~3.4us of dummy matmuls on a zeroed tile so the
            # HAM clock-gate un-throttles before the first real matmul lands.
            wrm = ipool.tile([128, 640], bf16, tag="wrm")
            nc.vector.memset(wrm[:], 0.0)
            ps0 = pspool.tile([128, 2, 512], f32, tag="ps")
            for w in range(8):
                nc.tensor.matmul(
                    ps0[:, w % 2, :],
                    wrm[:, 0:128],
                    wrm[:, 128:640],
                    start=True,
                    stop=True,
                )

            # Batches 0..NBF-1: host-prepared bf16 via the two HWDGE rings in
            # parallel, one DMA per batch so batch 0 lands ASAP (no cast, no
            # engine work). Batches NBF-15: SWDGE cast-DMA (int8->bf16 in the
            # DMA datapath) in 4 big chunks to amortize the ~2.7us per-DMA Q7
            # issue cost.
            for i in range(NBF):
                nc.sync.dma_start(out=a_host[i][:], in_=a_bf[i])
                nc.scalar.dma_start(out=b_host[i][:], in_=b_bf[i])
            nc.gpsimd.dma_start(out=a_mid[:], in_=a_v[:, NBF:8, :])
            nc.gpsimd.dma_start(out=b_mid[:], in_=b_v[:, NBF:8, :])
            nc.gpsimd.dma_start(out=a_hi[:], in_=a_v[:, 8:16, :])
            nc.gpsimd.dma_start(out=b_hi[:], in_=b_v[:, 8:16, :])

            di = 0  # drain-op index for ACT/DVE balancing
            for bi in range(BPC):
                at = a_of(bi)  # [128, 1024] k x m
                bt = b_of(bi)  # [128, 1024] k x n
                y_sb = opool.tile([128, MT, N], i8, tag="y")

                for mt in range(MT):  # one 2-bank psum tile per m-tile
                    ps = pspool.tile([128, 2, 512], f32, tag="ps")
                    for nh in range(2):
                        nc.tensor.matmul(
                            ps[:, nh, :],
                            at[:, mt * 128 : (mt + 1) * 128],
                            bt[:, nh * 512 : (nh + 1) * 512],
                            start=True,
                            stop=True,
                        )
                    dst = y_sb[:, mt, :]
                    src = ps.rearrange("p a b -> p (a b)")
                    # strict A/D interleave with 3 extra ACT tiles: 67 ACT vs
                    # 61 DVE balances measured 1122ns vs 1215ns per-op costs
                    if di % 2 == 0 or di in (21, 63, 105):
                        nc.scalar.activation(
                            out=dst,
                            in_=src,
                            func=mybir.ActivationFunctionType.Copy,
                            scale=float(alpha),
                        )
                    else:
                        nc.vector.tensor_scalar(
                            out=dst,
                            in0=src,
                            scalar1=float(alpha),
                            scalar2=None,
                            op0=mybir.AluOpType.mult,
                        )
                    di += 1
                    # half-batch stores (512 KiB) as soon as each half is
                    # drained, on the otherwise-idle sync ring only. NEVER on
                    # gpsimd: SWDGE descriptor generation thrashes SBUF ports
                    # and slows every concurrent ACT/DVE op ~20%. The last
                    # batch stores ever-finer so the final store after the
                    # last drain is only 128 KiB; mt6's store issues from the
                    # scalar ring in ACT's post-drain shadow.
                    yv = y[bi].rearrange("(t p) n -> p t n", p=128)
                    last = bi == BPC - 1
                    if mt == 3:
                        nc.sync.dma_start(out=yv[:, 0:4, :], in_=y_sb[:, 0:4, :])
                    elif not last and mt == 7:
                        nc.sync.dma_start(out=yv[:, 4:8, :], in_=y_sb[:, 4:8, :])
                    elif last and mt == 5:
                        nc.sync.dma_start(out=yv[:, 4:6, :], in_=y_sb[:, 4:6, :])
                    elif last and mt == 6:
                        nc.scalar.dma_start(out=yv[:, 6, :], in_=y_sb[:, 6, :])
                    elif last and mt == 7:
                        nc.sync.dma_start(out=yv[:, 7, :], in_=y_sb[:, 7, :])

    nc.compile()
    return nc


def _get_nc(alpha: float):
    key = float(alpha)
    if key not in _cache:
        _cache[key] = _build(key)
    return _cache[key]


def _shard_inputs(a, b):
    import ml_dtypes

    # host-side pre-transpose to [B, K, M] / [B, K, N]
    a_t = np.ascontiguousarray(a.transpose(0, 2, 1))
    b_t = np.ascontiguousarray(b.transpose(0, 2, 1))
    maps = []
    for c in range(N_CORES):
        at = a_t[c * BPC : (c + 1) * BPC]
        bt = b_t[c * BPC : (c + 1) * BPC]
        maps.append(
            {
                "a_t": at,
                "b_t": bt,
                "a_bf": at[:NBF].astype(ml_dtypes.bfloat16),
                "b_bf": bt[:NBF].astype(ml_dtypes.bfloat16),
            }
        )
    return maps


def kernel(a, b, alpha):
    from concourse.bass_utils import run_bass_kernel_spmd

    a = np.asarray(a)
    b = np.asarray(b)
    assert a.shape == (B, M, K) and a.dtype == np.int8
    assert b.shape == (B, N, K) and b.dtype == np.int8

    nc = _get_nc(float(alpha))
    in_maps = _shard_inputs(a, b)
    res = run_bass_kernel_spmd(nc, in_maps, list(range(N_CORES)))
    out = np.concatenate([r["y"] for r in res.results], axis=0)
    return out.astype(np.int8)
